# revision 66
# baseline (speedup 1.0000x reference)
"""CTLSTM (continuous-time LSTM, state re-init variant) Trainium2 kernel, v5.

Key insight: the reference re-initializes h/c/c_bar to zero every timestep, so
the 7H gate pre-activations depend ONLY on the event type (1001 distinct
embedding rows), not on the token. Outputs c, c_bar, go, gd are pure per-type
values; only h_d = go*tanh(c_bar + (c-c_bar)*exp(-gd*dur)) mixes in the
per-token duration.

Device (per core, tensor-parallel over H: core k owns H columns
[128k, 128k+128)):
  Phase 1 (~50us): G = embT @ W slices (bf16 matmuls, per-gate bias via a K=1
    ones-row matmul closing the PSUM group), then build the bf16 per-type
    DRAM table with 1KB rows [DIF | CB | GO | GD] (DIF = C-CB), type-major.
    The table is also shipped to the host (1MB) for outputs 1-4.
  Phase 2 (~150us): 8 waves x 2048 tokens: non-transpose SWDGE dma_gather
    (tokens land on partitions) -> g[128 tok, 16 chunk, 512]; exp(-gd*dur)
    fuses the duration in as a per-partition ACT scale; DVE computes h_d;
    bf16 h_d out (4MB). The wall is the gather's Q7 descriptor generation,
    a fixed ~8.3 ns/index (measured invariant across row size 0.5-8KB,
    dtype, DRAM/SBUF source, transpose) = ~135us for 16K tokens; transfers
    and all pointwise compute hide underneath it. (Prepare/trigger preps
    overlapping phase 1 would save another ~45us but the Tile scheduler's
    handling of user-synced preps raced unreliably; see session notes.)

Host: re-assembles h_d (transpose + f32 cast) and gathers outputs 1-4 from
the per-type tables by event id (replication of device-computed values).
"""

import os

import numpy as np

HIDDEN = 1024
TYPES = 1001
TPAD = 1024          # padded type count (8 m-tiles of 128)
B = 32
T = 512
NTOK = B * T         # 16384
NCORES = 8
KAUG = 1024          # contraction (bias handled by a K=1 matmul)
NGATES = 5           # i, z, o, ibar, d  (f, fbar unused by the reference)
GATE_ROWS = (0, 2, 3, 4, 6)  # row-group index of each used gate in W_rec/b_rec
NCOLS = NGATES * 128  # 640 gate columns per core
WAVE = 2048          # tokens per phase-2 wave
NWAVES = NTOK // WAVE
WCH = WAVE // 128    # chunks per wave (16)
KT = KAUG // 128     # 8
PREP_WINDOW = 3      # max untriggered SWDGE preps (ring capacity)

LAST_RESULTS = None
_CACHED_NC = None


def _build_nc():
    import concourse.mybir as mybir
    from concourse import bacc
    from concourse.tile import TileContext

    dt = mybir.dt
    AF = mybir.ActivationFunctionType
    f32 = dt.float32
    bf16 = dt.bfloat16

    nc = bacc.Bacc("TRN2", target_bir_lowering=False, debug=False,
                   dynamic_dma_scratch_size=32768, num_swdge_queues=2)

    et_d = nc.dram_tensor("et", [128, KT, TPAD], bf16, kind="ExternalInput")
    wt_d = nc.dram_tensor("wt", [128, KT, NCOLS], bf16, kind="ExternalInput")
    bias_d = nc.dram_tensor("bias", [1, NCOLS], bf16, kind="ExternalInput")
    idx_d = nc.dram_tensor("idx", [128, NTOK // 16], dt.int16, kind="ExternalInput")
    dur_d = nc.dram_tensor("durneg", [128, NTOK // 128], f32, kind="ExternalInput")
    hd_d = nc.dram_tensor("hd", [128, NTOK // 128, 128], bf16, kind="ExternalOutput")
    # per-type table, 1KB rows [DIF | CB | GO | GD]; doubles as gather source
    tbl_d = nc.dram_tensor("tbl", [TPAD, 512], bf16, kind="ExternalOutput")

    from concourse import library_config

    with TileContext(nc) as tc:
        # load the dma_gather ucode overlay while nothing is outstanding —
        # the auto-inserted reload would otherwise wait for DMA quiescence
        # right before the first gather (~50us in)
        nc.gpsimd.load_library(library_config.mlp)
        with tc.tile_pool(name="const", bufs=1) as cpool:
            # et/wt first: they gate the phase-1 matmuls; idx isn't needed
            # until the first gather (~47us in)
            et_sb = cpool.tile([128, KT, TPAD], bf16, tag="et")
            wt_sb = cpool.tile([128, KT, NCOLS], bf16, tag="wt")
            for kt in range(KT):
                nc.sync.dma_start(out=et_sb[:, kt, :], in_=et_d[:, kt, :])
                nc.sync.dma_start(out=wt_sb[:, kt, :], in_=wt_d[:, kt, :])
            bias_sb = cpool.tile([1, NCOLS], bf16, tag="bias")
            nc.sync.dma_start(out=bias_sb[:], in_=bias_d[:])
            idx_sb = cpool.tile([128, NTOK // 16], dt.int16, tag="idx")
            nc.sync.dma_start(out=idx_sb[:], in_=idx_d[:])
            dur_sb = cpool.tile([128, NTOK // 128], f32, tag="dur")
            nc.sync.dma_start(out=dur_sb[:], in_=dur_d[:])
            ones_bf = cpool.tile([1, 128], bf16, tag="onesb")
            nc.vector.memset(ones_bf[:], 1.0)

            gd_all = cpool.tile([128, KT, 128], f32, tag="gdall")
            gd_exp = cpool.tile([128, KT, 128], f32, tag="gdexp")
            gd_out = cpool.tile([128, KT, 128], bf16, tag="gdout")

            # ---- phase 1: per-type gate table ----------------------------
            with (
                tc.tile_pool(name="p1psum", bufs=3, space="PSUM") as ppool,
                tc.tile_pool(name="p1sb", bufs=3) as epool,
                tc.tile_pool(name="wave", bufs=5) as wpool,
                tc.tile_pool(name="scr", bufs=2) as spool,
                tc.tile_pool(name="dram", bufs=1, space="DRAM") as dpool,
            ):
                # internal gather table; real type t lives at row t+2 (the
                # 2 pad rows keep the layout compatible with the host +2
                # index bias)
                table_a = dpool.tile([TPAD + 2, 512], bf16, tag="tableg")
                for m in range(TPAD // 128):
                    psA = ppool.tile([128, 384], f32, tag="psA")  # gi|gz|go
                    psB = ppool.tile([128, 256], f32, tag="psB")  # gib|gd
                    for kt in range(KT):
                        lhs = et_sb[:, kt, m * 128:(m + 1) * 128]
                        first = kt == 0
                        nc.tensor.matmul(psA[:, :], lhs, wt_sb[:, kt, 0:384],
                                         start=first, stop=False)
                        nc.tensor.matmul(psB[:, :], lhs, wt_sb[:, kt, 384:640],
                                         start=first, stop=False)
                    nc.tensor.matmul(psA[:, :], ones_bf[0:1, :],
                                     bias_sb[0:1, 0:384], start=False, stop=True)
                    nc.tensor.matmul(psB[:, :], ones_bf[0:1, :],
                                     bias_sb[0:1, 384:640], start=False, stop=True)
                    merged = epool.tile([128, 384], bf16, tag="merged")
                    gi = epool.tile([128, 128], f32, tag="gi")
                    gz = epool.tile([128, 128], f32, tag="gz")
                    gib = epool.tile([128, 128], f32, tag="gib")
                    dif = epool.tile([128, 128], f32, tag="dif")
                    nc.scalar.activation(out=gi[:], in_=psA[:, 0:128], func=AF.Sigmoid)
                    nc.scalar.activation(out=gz[:], in_=psA[:, 128:256], func=AF.Tanh)
                    nc.scalar.activation(out=merged[:, 256:384], in_=psA[:, 256:384],
                                         func=AF.Sigmoid)
                    nc.scalar.activation(out=gib[:], in_=psB[:, 0:128], func=AF.Sigmoid)
                    nc.vector.tensor_copy(out=gd_all[:, m, :], in_=psB[:, 128:256])
                    nc.vector.tensor_sub(dif[:], gi[:], gib[:])
                    nc.vector.tensor_mul(merged[:, 0:128], dif[:], gz[:])
                    nc.vector.tensor_mul(merged[:, 128:256], gib[:], gz[:])
                    nc.sync.dma_start(
                        out=table_a[2 + m * 128:2 + (m + 1) * 128, 0:384],
                        in_=merged[:])
                # softplus(gd) = Ln(1 + Exp(gd)), batched to limit ACT table loads
                nc.scalar.activation(out=gd_exp[:], in_=gd_all[:], func=AF.Exp)
                nc.scalar.activation(out=gd_out[:], in_=gd_exp[:], func=AF.Ln,
                                     bias=1.0)
                nc.sync.dma_start(
                    out=table_a[2:TPAD + 2, 384:512].rearrange(
                        "(m p) e -> p m e", p=128),
                    in_=gd_out[:])
                # ship the table to the host for outputs 1-4 (overlaps phase 2)
                nc.sync.dma_start(out=tbl_d[:], in_=table_a[2:TPAD + 2, :])

                # ---- phase 2: gather + pointwise -------------------------
                for w in range(NWAVES):
                    g = wpool.tile([128, WCH, 512], bf16, tag="g", name=f"g{w}")
                    nc.gpsimd.dma_gather(
                        g[:],
                        table_a[:],
                        idx_sb[:, w * (WAVE // 16):(w + 1) * (WAVE // 16)],
                        WAVE,
                        WAVE,
                        512,
                        single_packet=False,
                    )
                    te = spool.tile([128, WCH, 128], f32, tag="te")
                    for c in range(WCH):
                        wc = w * WCH + c
                        nc.scalar.activation(
                            out=te[:, c, :], in_=g[:, c, 384:512], func=AF.Exp,
                            scale=dur_sb[:, wc:wc + 1])
                    td = spool.tile([128, WCH, 128], f32, tag="td")
                    nc.vector.tensor_mul(td[:], g[:, :, 0:128], te[:])
                    nc.vector.tensor_add(td[:], td[:], g[:, :, 128:256])
                    nc.scalar.activation(out=te[:], in_=td[:], func=AF.Tanh)
                    hdw = wpool.tile([128, WCH, 128], bf16, tag="hdw")
                    nc.vector.tensor_mul(hdw[:], g[:, :, 256:384], te[:])
                    nc.sync.dma_start(
                        out=hd_d[:, w * WCH:(w + 1) * WCH, :], in_=hdw[:])

    nc.compile()
    return nc


def _ensure_ntff_hook():
    """The agent image's antenv lacks axon_hooks; shim it and register the
    ctypes NTFF profiling hook so trace=True works under axon."""
    import sys
    import types

    try:
        from antenv.axon_hooks import get_axon_ntff_profile_hook  # noqa: F401
        return
    except ImportError:
        pass
    try:
        import antenv
    except ImportError:
        return
    mod = types.ModuleType("antenv.axon_hooks")
    state = {"hook": None}
    mod.set_axon_ntff_profile_hook = lambda h: state.__setitem__("hook", h)
    mod.get_axon_ntff_profile_hook = lambda: state["hook"]
    sys.modules["antenv.axon_hooks"] = mod
    antenv.axon_hooks = mod
    try:
        from trn_agent_boot.trn_boot import _ntff_profile_via_ctypes

        hook = _ntff_profile_via_ctypes("/opt/axon/libaxon_pjrt.so")
        if hook is not None:
            mod.set_axon_ntff_profile_hook(hook)
    except Exception:
        pass


def kernel(event_seqs, duration_seqs, emb_table, W_rec, b_rec):
    global LAST_RESULTS, _CACHED_NC
    import ml_dtypes
    from concourse.bass_utils import run_bass_kernel_spmd

    bf16 = ml_dtypes.bfloat16
    ev = np.asarray(event_seqs)
    dur = np.asarray(duration_seqs, dtype=np.float32)
    emb = np.asarray(emb_table, dtype=np.float32)
    W = np.asarray(W_rec, dtype=np.float32)
    b = np.asarray(b_rec, dtype=np.float32)

    # ---- host-side input marshaling (sharding) -----------------------------
    embT = np.zeros((HIDDEN, TPAD), np.float32)
    embT[:, :TYPES] = emb.T
    et = np.ascontiguousarray(
        embT.reshape(KT, 128, TPAD).transpose(1, 0, 2)).astype(bf16)

    # +2: the device table has 2 never-written pad rows at the front
    ev_tok = (ev.T.reshape(-1) + 2).astype(np.int16)    # token t*32+b -> row
    # idx i at [i%16, i//16], replicated across the 8 GPSIMD core stripes
    idx = np.tile(ev_tok.reshape(-1, 16).T, (8, 1)).astype(np.int16)

    # token chunk*128+p -> per-partition ACT scale column
    durneg = np.ascontiguousarray((-dur.T.reshape(-1)).reshape(-1, 128).T)

    in_maps = []
    for k in range(NCORES):
        h0 = 128 * k
        wt = np.zeros((HIDDEN, NCOLS), np.float32)
        bias = np.zeros((1, NCOLS), np.float32)
        for g5, g7 in enumerate(GATE_ROWS):
            rows = slice(g7 * HIDDEN + h0, g7 * HIDDEN + h0 + 128)
            wt[:, g5 * 128:(g5 + 1) * 128] = W[rows, :HIDDEN].T
            bias[0, g5 * 128:(g5 + 1) * 128] = b[rows]
        wt = np.ascontiguousarray(
            wt.reshape(KT, 128, NCOLS).transpose(1, 0, 2)).astype(bf16)
        in_maps.append({"et": et, "wt": wt, "bias": bias.astype(bf16),
                        "idx": idx, "durneg": durneg})

    if _CACHED_NC is None:
        _CACHED_NC = _build_nc()
    nc = _CACHED_NC

    trace = os.environ.get("KERNEL_TRACE", "") not in ("", "0")
    if trace:
        _ensure_ntff_hook()
    res = run_bass_kernel_spmd(nc, in_maps, list(range(NCORES)), trace=trace)
    LAST_RESULTS = res

    # ---- host-side output assembly ----------------------------------------
    full = np.empty((5, NTOK, HIDDEN), np.float32)
    ev_flat = ev.T.reshape(-1).astype(np.int64)
    qtbl = np.empty((4, TPAD, HIDDEN), np.float32)  # C, CB, GO, GD
    for k in range(NCORES):
        sl = slice(128 * k, 128 * (k + 1))
        o = res.results[k]["hd"]  # [128, chunks, 128] bf16
        full[0].reshape(NTOK // 128, 128, HIDDEN)[:, :, sl] = (
            o.transpose(1, 0, 2))
        tbl = res.results[k]["tbl"].astype(np.float32)  # [1024, 512]
        qtbl[0, :, sl] = tbl[:, 0:128] + tbl[:, 128:256]  # C = DIF + CB
        qtbl[1, :, sl] = tbl[:, 128:256]
        qtbl[2, :, sl] = tbl[:, 256:384]
        qtbl[3, :, sl] = tbl[:, 384:512]
    for s in range(4):
        full[s + 1] = qtbl[s][ev_flat]
    return full.reshape(5, T, B, HIDDEN)
